# revision 15
# baseline (speedup 1.0000x reference)
"""Trainium2 Bass kernel for nn_DiffRankNet (retrieval_knn).

Strategy (8 NeuronCores, zero mid-kernel feature collectives):
  Launch A  (core = (side s, quarter q)): fp16 cdist score rows for the
    core's N/4 nodes + exact top-16 via max8/max_index/match_replace, plus
    the A1 = x @ W1[:, q-slice] projection.  Outputs per core: knn indices
    [N/4, 16] and A1 slice [N, HID/4].
  Host: pure index bookkeeping — degree vectors, gather lists, one-hot
    weights, balanced dst-tiles, the topology vector b~ (layer 2 of the
    HGNN collapses to b~^T relu(raw) w2s because the final output only
    needs mean().sum()).
  Launch B  (core = (side s, hid-slice v)): dma_gather edge neighborhoods
    of A1-slice rows, prefix-sum matmuls -> c rows, gather c rows by
    dst-sorted membership lists, one-hot matmuls accumulate raw out1^T
    tiles, relu, dot with w2s and b~, one tiny [1,2] AllReduce + sigmoid.
"""

import os
from contextlib import ExitStack
from dataclasses import dataclass

import numpy as np
import ml_dtypes

import concourse.bass as bass
import concourse.mybir as mybir
from concourse import bacc
import concourse.tile as tile
from concourse import library_config
from concourse.bass_utils import run_bass_kernel_spmd

f32 = mybir.dt.float32
bf16 = mybir.dt.bfloat16
fp16 = mybir.dt.float16
i16 = mybir.dt.int16
u32 = mybir.dt.uint32

KS = (5, 10, 15)
NQ = 4          # quarters / hid-slices per side
NCORES = 8
PAD_SLOT = 15   # 16th gather slot per triple is padding
GS = 16         # gather slots per triple


@dataclass(frozen=True)
class Cfg:
    N: int = 4096
    C: int = 1024
    HID: int = 512
    FEAT: int = 128

    @property
    def RQ(self):  # cdist rows per core
        return self.N // NQ

    @property
    def RT(self):  # cdist row-tile (partition) size
        return min(128, self.RQ)

    @property
    def SL(self):  # hid cols per slice-core
        return self.HID // NQ

    @property
    def KC(self):  # contraction chunks of x
        return self.C // 128

    @property
    def CCW(self):  # cdist psum col-tile width
        return min(512, self.N)

    @property
    def G1(self):  # stage-1 gather entries
        return self.N * GS

    @property
    def B1(self):  # stage-1 gather batch (entries; HW-safe dma_gather size)
        return min(1024, self.G1)

    @property
    def CROWS(self):
        return self.N * 4

    @property
    def S2T(self):  # dst tiles
        return self.N // 128

    @property
    def KCH(self):  # scatter chunks per dst tile
        return 17

    @property
    def S2N(self):
        return self.S2T * self.KCH * 128


CFG = Cfg()


# --------------------------------------------------------------------------
# Launch A: cdist + topk + A1 slice
# --------------------------------------------------------------------------
def build_launch_a(cfg: Cfg):
    nc = bacc.Bacc("TRN2", target_bir_lowering=False, debug=False,
                   num_devices=NCORES)
    N, C, KC, RT, CCW = cfg.N, cfg.C, cfg.KC, cfg.RT, cfg.CCW
    IT = cfg.RQ // RT          # row tiles per core
    CCn = N // CCW             # cdist col tiles
    OT = N // 128              # A1 row tiles

    xt_d = nc.dram_tensor("xt", [C, N], fp16, kind="ExternalInput")
    xmyt_d = nc.dram_tensor("xmyt", [C, cfg.RQ], fp16, kind="ExternalInput")
    w1s_d = nc.dram_tensor("w1s", [C, cfg.SL], fp16, kind="ExternalInput")
    idx_o = nc.dram_tensor("idx_out", [cfg.RQ, 16], u32, kind="ExternalOutput")
    a1_o = nc.dram_tensor("a1_out", [N, cfg.SL], fp16, kind="ExternalOutput")

    with tile.TileContext(nc) as tc, ExitStack() as ctx:
        xt_p = ctx.enter_context(tc.tile_pool(name="xt", bufs=KC))
        xmy_p = ctx.enter_context(tc.tile_pool(name="xmy", bufs=KC))
        w1_p = ctx.enter_context(tc.tile_pool(name="w1", bufs=KC))
        const_p = ctx.enter_context(tc.tile_pool(name="const", bufs=1))
        sq_p = ctx.enter_context(tc.tile_pool(name="sq", bufs=2))
        d2_p = ctx.enter_context(tc.tile_pool(name="d2", bufs=2))
        top_p = ctx.enter_context(tc.tile_pool(name="top", bufs=2))
        a1s_p = ctx.enter_context(tc.tile_pool(name="a1s", bufs=3))
        d2ps_p = ctx.enter_context(tc.tile_pool(name="d2ps", bufs=6, space="PSUM"))
        aux_ps = ctx.enter_context(tc.tile_pool(name="auxps", bufs=2, space="PSUM"))

        xt = []
        xmy = []
        w1 = []
        for kc in range(KC):
            t = xt_p.tile([128, N], fp16)
            nc.sync.dma_start(t[:], xt_d.ap()[kc * 128:(kc + 1) * 128, :])
            xt.append(t)
            t2 = xmy_p.tile([128, cfg.RQ], fp16)
            nc.sync.dma_start(t2[:], xmyt_d.ap()[kc * 128:(kc + 1) * 128, :])
            xmy.append(t2)
            t3 = w1_p.tile([128, cfg.SL], fp16)
            nc.sync.dma_start(t3[:], w1s_d.ap()[kc * 128:(kc + 1) * 128, :])
            w1.append(t3)

        ones_col = const_p.tile([128, 1], fp16)
        nc.vector.memset(ones_col[:], 1.0)
        auglhs = const_p.tile([128, 128], fp16)
        nc.vector.memset(auglhs[:], 0.0)
        nc.vector.memset(auglhs[0:2, :], 1.0)
        aug = const_p.tile([128, N], fp16)
        nc.vector.memset(aug[:], 0.0)

        # n_j rows (hi/lo of -n/2) folded into the contraction
        for cc in range(CCn):
            n_ps = aux_ps.tile([1, CCW], f32, tag="aux")
            for kc in range(KC):
                sq = sq_p.tile([128, CCW], fp16)
                nc.scalar.square(sq[:], xt[kc][:, cc * CCW:(cc + 1) * CCW])
                nc.tensor.matmul(n_ps[:], lhsT=ones_col[:], rhs=sq[:],
                                 start=(kc == 0), stop=(kc == KC - 1))
            hi_t = sq_p.tile([1, CCW], fp16, tag="hilo")
            lo_t = sq_p.tile([1, CCW], fp16, tag="hilo")
            nc.scalar.mul(hi_t[:], n_ps[:], -0.5)
            nc.vector.scalar_tensor_tensor(
                out=lo_t[:], in0=n_ps[:], scalar=-0.5, in1=hi_t[:],
                op0=mybir.AluOpType.mult, op1=mybir.AluOpType.subtract)
            nc.sync.dma_start(aug[0:1, cc * CCW:(cc + 1) * CCW], hi_t[:])
            nc.sync.dma_start(aug[1:2, cc * CCW:(cc + 1) * CCW], lo_t[:])

        # cdist + topk per row tile.  kc-outer over cc-halves so the
        # stationary operand loads once per (it, kc, half) instead of per
        # (it, cc, kc); the half's psum tiles accumulate in parallel.
        HCC = max(1, CCn // 2)
        for it in range(IT):
            d2 = d2_p.tile([RT, N], f32)
            for h in range(CCn // HCC if CCn >= HCC else 1):
                pss = []
                for _ci in range(HCC):
                    ps_t = d2ps_p.tile([RT, CCW], f32, tag="d2ps")
                    pss.append(ps_t)
                for kc in range(KC):
                    for ci in range(HCC):
                        cc = h * HCC + ci
                        nc.tensor.matmul(
                            pss[ci][:],
                            lhsT=xmy[kc][:, it * RT:(it + 1) * RT],
                            rhs=xt[kc][:, cc * CCW:(cc + 1) * CCW],
                            start=(kc == 0), stop=False)
                for ci in range(HCC):
                    cc = h * HCC + ci
                    nc.tensor.matmul(
                        pss[ci][:], lhsT=auglhs[:, :RT],
                        rhs=aug[:, cc * CCW:(cc + 1) * CCW],
                        start=False, stop=True)
                    nc.scalar.copy(d2[:, cc * CCW:(cc + 1) * CCW],
                                   pss[ci][:])
            mx1 = top_p.tile([RT, 8], f32, tag="mx")
            nc.vector.max(out=mx1[:], in_=d2[:])
            ix1 = top_p.tile([RT, 8], u32, tag="ix")
            nc.vector.max_index(ix1[:], mx1[:], d2[:])
            nc.vector.match_replace(out=d2[:], in_to_replace=mx1[:],
                                    in_values=d2[:], imm_value=-3.0e38)
            mx2 = top_p.tile([RT, 8], f32, tag="mx")
            nc.vector.max(out=mx2[:], in_=d2[:])
            ix2 = top_p.tile([RT, 8], u32, tag="ix")
            nc.vector.max_index(ix2[:], mx2[:], d2[:])
            idxt = top_p.tile([RT, 16], u32, tag="idxt")
            nc.vector.tensor_copy(idxt[:, 0:8], ix1[:])
            nc.vector.tensor_copy(idxt[:, 8:16], ix2[:])
            nc.sync.dma_start(idx_o.ap()[it * RT:(it + 1) * RT, :], idxt[:])

        # A1 slice
        for ot in range(OT):
            aps = aux_ps.tile([128, cfg.SL], f32, tag="aux")
            for kc in range(KC):
                nc.tensor.matmul(aps[:],
                                 lhsT=xt[kc][:, ot * 128:(ot + 1) * 128],
                                 rhs=w1[kc][:],
                                 start=(kc == 0), stop=(kc == KC - 1))
            a1sb = a1s_p.tile([128, cfg.SL], fp16)
            nc.scalar.copy(a1sb[:], aps[:])
            nc.sync.dma_start(a1_o.ap()[ot * 128:(ot + 1) * 128, :], a1sb[:])

    nc.compile()
    return nc


# --------------------------------------------------------------------------
# Launch B: hyconv sandwich on A1 slices + scalar tail
# --------------------------------------------------------------------------
def build_launch_b(cfg: Cfg):
    nc = bacc.Bacc("TRN2", target_bir_lowering=False, debug=False,
                   num_devices=NCORES)
    N, SL = cfg.N, cfg.SL
    G1, B1 = cfg.G1, cfg.B1
    NB1 = G1 // B1                 # stage-1 gather batches
    NCH1 = B1 // 128               # chunks per stage-1 batch
    NBG1 = NCH1 // 4               # psum bank groups per batch
    RPB = B1 // 4                  # c rows per batch
    S2T, KCH = cfg.S2T, cfg.KCH
    TPB2 = 2 if S2T >= 2 else 1    # dst tiles per stage-2 gather batch
    GB2 = TPB2 * KCH * 128
    NB2 = S2T // TPB2

    z1_d = nc.dram_tensor("z1", [N + 1, SL], fp16, kind="ExternalInput")
    gidx_d = nc.dram_tensor("gidx", [128, G1 // 16], i16, kind="ExternalInput")
    gw_d = nc.dram_tensor("gw", [128, G1 // 128], f32, kind="ExternalInput")
    umask_d = nc.dram_tensor("umask", [128, 32], fp16, kind="ExternalInput")
    ssrc_d = nc.dram_tensor("ssrc", [128, cfg.S2N // 16], i16, kind="ExternalInput")
    sdst_d = nc.dram_tensor("sdst", [128, S2T * KCH], f32, kind="ExternalInput")
    iota_d = nc.dram_tensor("iota_f", [128, 128], fp16, kind="ExternalInput")
    w2t_d = nc.dram_tensor("w2t", [cfg.FEAT, 128], fp16, kind="ExternalInput")
    bperm_d = nc.dram_tensor("bperm", [1, N], f32, kind="ExternalInput")

    res_o = nc.dram_tensor("res", [1, 1], f32, kind="ExternalOutput")

    c_hbm = nc.dram_tensor("c_hbm", [cfg.CROWS + 1, 128], fp16)

    with tile.TileContext(nc) as tc, ExitStack() as ctx:
        const_p = ctx.enter_context(tc.tile_pool(name="const", bufs=1))
        g1_p = ctx.enter_context(tc.tile_pool(name="g1", bufs=2))
        t1w_p = ctx.enter_context(tc.tile_pool(name="t1w", bufs=2))
        stg_p = ctx.enter_context(tc.tile_pool(name="stg", bufs=2))
        g2_p = ctx.enter_context(tc.tile_pool(name="g2", bufs=2))
        t2_p = ctx.enter_context(tc.tile_pool(name="t2", bufs=3))
        rl_p = ctx.enter_context(tc.tile_pool(name="rl", bufs=2))
        fin_p = ctx.enter_context(tc.tile_pool(name="fin", bufs=1))
        cps_p = ctx.enter_context(tc.tile_pool(name="cps", bufs=3, space="PSUM"))
        ops_p = ctx.enter_context(tc.tile_pool(name="ops", bufs=3, space="PSUM"))
        aux_ps = ctx.enter_context(tc.tile_pool(name="auxps", bufs=2, space="PSUM"))

        nc.gpsimd.load_library(library_config.mlp)

        # constant-ish inputs
        gidx = const_p.tile([128, G1 // 16], i16)
        nc.sync.dma_start(gidx[:], gidx_d.ap())
        gw = const_p.tile([128, G1 // 128], f32)
        nc.sync.dma_start(gw[:], gw_d.ap())
        umask = const_p.tile([128, 32], fp16)
        nc.sync.dma_start(umask[:], umask_d.ap())
        ssrc = const_p.tile([128, cfg.S2N // 16], i16)
        nc.sync.dma_start(ssrc[:], ssrc_d.ap())
        sdst = const_p.tile([128, S2T * KCH], f32)
        nc.sync.dma_start(sdst[:], sdst_d.ap())
        iota = const_p.tile([128, 128], fp16)
        nc.sync.dma_start(iota[:], iota_d.ap())
        w2t = const_p.tile([cfg.FEAT, 128], fp16)
        nc.sync.dma_start(w2t[:], w2t_d.ap())
        bperm = const_p.tile([1, N], f32)
        nc.sync.dma_start(bperm[:], bperm_d.ap())

        ones_col = const_p.tile([128, 1], fp16)
        nc.vector.memset(ones_col[:], 1.0)

        # zero pad row of c storage
        zr = const_p.tile([1, 128], fp16)
        nc.vector.memset(zr[:], 0.0)
        nc.sync.dma_start(c_hbm.ap()[cfg.CROWS:cfg.CROWS + 1, :], zr[:])

        # w2s column = W2[v-slice, :] @ 1  (lhsT = W2T slice)
        w2s_ps = aux_ps.tile([128, 1], f32, tag="aux")
        nc.tensor.matmul(w2s_ps[:], lhsT=w2t[:], rhs=ones_col[:cfg.FEAT, :],
                         start=True, stop=True)
        w2s = const_p.tile([128, 1], fp16)
        nc.vector.tensor_copy(w2s[:], w2s_ps[:])

        # ---- stage 1: edge-neighborhood gather + prefix-sum matmuls ----
        for b in range(NB1):
            g = g1_p.tile([128, NCH1, 128], fp16)
            nc.gpsimd.dma_gather(
                out_ap=g[:], in_ap=z1_d.ap(),
                idxs_ap=gidx[:, b * (B1 // 16):(b + 1) * (B1 // 16)],
                num_idxs=B1, num_idxs_reg=B1, elem_size=128)
            t1w = t1w_p.tile([128, NCH1, 32], fp16)
            nc.vector.tensor_tensor(
                out=t1w[:],
                in0=umask[:].rearrange("p (c m) -> p c m", c=1)
                    .to_broadcast([128, NCH1, 32]),
                in1=gw[:, b * NCH1:(b + 1) * NCH1]
                    .rearrange("p (c m) -> p c m", m=1)
                    .to_broadcast([128, NCH1, 32]),
                op=mybir.AluOpType.mult)
            stg = stg_p.tile([128, NBG1 * 128], fp16)
            for bg in range(NBG1):
                cps = cps_p.tile([128, 128], f32)
                for j in range(4):
                    c = bg * 4 + j
                    nc.tensor.matmul(
                        cps[32 * j:32 * j + 32, :],
                        lhsT=t1w[:, c, :], rhs=g[:, c, :],
                        start=True, stop=True,
                        tile_position=(0, 32 * j))
                nc.scalar.copy(stg[:, bg * 128:(bg + 1) * 128], cps[:])
            nc.sync.dma_start(
                c_hbm.ap()[b * RPB:(b + 1) * RPB, :]
                    .rearrange("(g p) f -> p g f", p=128),
                stg[:].rearrange("p (g f) -> p g f", f=128))

        # ---- stage 2: dst-sorted membership gather + one-hot matmuls ----
        yacc = fin_p.tile([1, S2T], f32)
        yscr = fin_p.tile([1, 128], f32)
        S2CH = S2T * KCH
        GCH = 8                     # chunks per gather batch (1024 idxs)
        g2 = None
        ops = None
        for ch in range(S2CH):
            if ch % GCH == 0:
                nch = min(GCH, S2CH - ch)
                g2 = g2_p.tile([128, GCH, 128], fp16, tag="g2")
                nc.gpsimd.dma_gather(
                    out_ap=g2[:, 0:nch, :], in_ap=c_hbm.ap(),
                    idxs_ap=ssrc[:, ch * 8:(ch + nch) * 8],
                    num_idxs=nch * 128, num_idxs_reg=nch * 128,
                    elem_size=128)
            d, j = divmod(ch, KCH)
            if j == 0:
                ops = ops_p.tile([128, 128], f32, tag="ops")
            t2 = t2_p.tile([128, 128], fp16)
            nc.vector.tensor_scalar(
                out=t2[:], in0=iota[:],
                scalar1=sdst[:, ch:ch + 1], scalar2=None,
                op0=mybir.AluOpType.is_equal)
            nc.tensor.matmul(ops[:], lhsT=g2[:, ch % GCH, :], rhs=t2[:],
                             start=(j == 0), stop=(j == KCH - 1))
            if j == KCH - 1:
                rl = rl_p.tile([128, 128], fp16)
                nc.scalar.activation(rl[:], ops[:],
                                     mybir.ActivationFunctionType.Relu)
                rps = aux_ps.tile([1, 128], f32, tag="aux")
                nc.tensor.matmul(rps[:], lhsT=w2s[:], rhs=rl[:],
                                 start=True, stop=True)
                nc.vector.scalar_tensor_tensor(
                    out=yscr[:], in0=rps[:], scalar=1.0,
                    in1=bperm[:, d * 128:(d + 1) * 128],
                    op0=mybir.AluOpType.mult, op1=mybir.AluOpType.mult,
                    accum_out=yacc[0:1, d:d + 1])

        # ---- tail: per-core partial scalar out (host combines) ----
        ysum = fin_p.tile([1, 1], f32)
        nc.vector.tensor_reduce(out=ysum[:], in_=yacc[:],
                                axis=mybir.AxisListType.X,
                                op=mybir.AluOpType.add)
        nc.sync.dma_start(res_o.ap(), ysum[:])

    nc.compile()
    return nc


# --------------------------------------------------------------------------
# Host-side index bookkeeping between launches
# --------------------------------------------------------------------------
def _wrap16(arr):
    """idx j -> [j % 16, j // 16], tiled to 128 partitions."""
    a = np.asarray(arr, dtype=np.int16).reshape(-1, 16).T
    return np.tile(a, (8, 1))


def host_lists(cfg: Cfg, idx):
    """idx: [N, 15] int64 neighbor indices (rank order). Returns dict of
    launch-B input arrays (except z1/w2t/smask) for one side."""
    N = cfg.N
    K0, K1, K2 = KS
    r = np.arange(15)
    # degree vectors
    Dv = np.zeros(N, np.int64)
    for k in KS:
        Dv += np.bincount(idx[:, :k].ravel(), minlength=N)
    dv = (1.0 / np.sqrt(Dv)).astype(np.float64)

    # u_a(r) = sum_{lvl >= a, KS[lvl] > r} 1/KS[lvl]
    u = np.zeros((3, 16), np.float64)
    for a in range(3):
        for rr in range(15):
            u[a, rr] = sum(1.0 / k for li, k in enumerate(KS)
                           if li >= a and k > rr)
    # umask [128, 32]
    umask = np.zeros((128, 32), np.float64)
    for p in range(128):
        t, rr = p // 16, p % 16
        if rr < 15:
            for a in range(3):
                umask[p, t * 4 + a] = u[a, rr]
    umask = umask.astype(np.float16)

    # stage-1 gather list + per-entry dv weight
    gl = np.full((N, GS), N, np.int64)
    gl[:, :15] = idx[:, :15]
    gidx = _wrap16(gl.ravel())
    gwv = np.zeros(N * GS, np.float64)
    gwv.reshape(N, GS)[:, :15] = dv[idx[:, :15]]
    gw = gwv.reshape(-1, 128).T.astype(np.float32)  # [128, G1//128]

    # c-row id for (e, a)
    TPB = cfg.B1 // 16
    def crow(e, a):
        eb, el = e // TPB, e % TPB
        c = el // 8
        return eb * (TPB * 4) + (c // 4) * 128 + (c % 4) * 32 + (e % 8) * 4 + a

    # b~ = (H g) * dv^2 with g = de * (H^T dv)
    S = np.add.reduceat(dv[idx[:, :15]], [0, K0, K1], axis=1).cumsum(1)
    # S[:, l] = sum_{r < KS[l]} dv[idx]; columns: [0:K0], [K0:K1], [K1:15]
    glev = S / np.array(KS, np.float64)[None, :]     # g_(e,lvl)
    gcomb = np.zeros((N, 15), np.float64)
    for rr in range(15):
        lvls = [li for li, k in enumerate(KS) if k > rr]
        gcomb[:, rr] = glev[:, lvls].sum(1)
    Hg = np.bincount(idx[:, :15].ravel(), weights=gcomb.ravel(), minlength=N)
    bt = Hg * dv * dv

    # balanced dst tiles: greedy LPT under exact-128 capacity
    mcnt = np.bincount(idx[:, :15].ravel(), minlength=N)
    order = np.argsort(-mcnt, kind="stable")
    S2T = cfg.S2T
    tile_of = np.zeros(N, np.int64)
    loc_of = np.zeros(N, np.int64)
    tl_fill = np.zeros(S2T, np.int64)
    tl_cnt = np.zeros(S2T, np.int64)
    INF = 1 << 60
    cost = np.zeros(S2T, np.int64)
    for node in order:
        d = int(np.argmin(np.where(tl_fill < 128, tl_cnt, INF)))
        tile_of[node] = d
        loc_of[node] = tl_fill[d]
        tl_fill[d] += 1
        tl_cnt[d] += mcnt[node]
    assert (tl_fill == 128).all()
    cap = cfg.KCH * 128
    assert tl_cnt.max() <= cap, f"dst tile overflow: {tl_cnt.max()} > {cap}"

    # membership entries
    e_arr = np.repeat(np.arange(N), 15)
    r_arr = np.tile(np.arange(15), N)
    dst = idx[:, :15].ravel()
    buck = (r_arr >= K0).astype(np.int64) + (r_arr >= K1)
    src = crow(e_arr, buck)
    dtile = tile_of[dst]
    dloc = loc_of[dst]

    ssrc_full = np.full(cfg.S2N, cfg.CROWS, np.int64)
    sdst_full = np.zeros(cfg.S2N, np.int64)
    for d in range(S2T):
        sel = np.flatnonzero(dtile == d)
        base = d * cap
        ssrc_full[base:base + sel.size] = src[sel]
        sdst_full[base:base + sel.size] = dloc[sel]
    # wrap ssrc per 1024-idx gather batch (ragged tail allowed)
    GB2 = 1024
    parts = [_wrap16(ssrc_full[g:g + GB2])
             for g in range(0, cfg.S2N, GB2)]
    ssrc = np.concatenate(parts, axis=1)
    sdst = sdst_full.reshape(-1, 128).T.astype(np.float32)  # [128, S2T*KCH]

    # b~ permuted to (tile, local) order
    bperm = np.zeros(N, np.float64)
    bperm[tile_of * 128 + loc_of] = bt
    return dict(gidx=gidx, gw=gw, umask=umask, ssrc=ssrc.astype(np.int16),
                sdst=sdst, bperm=bperm.astype(np.float32)[None, :])


# --------------------------------------------------------------------------
# Top-level kernel
# --------------------------------------------------------------------------
_NC_CACHE = {}
LAST_EXEC_NS = None
LAST_EXEC_PARTS = []


def _get_programs(cfg: Cfg):
    key = (cfg.N, cfg.C, cfg.HID, cfg.FEAT)
    if key not in _NC_CACHE:
        _NC_CACHE[key] = (build_launch_a(cfg), build_launch_b(cfg))
    return _NC_CACHE[key]


def _run(nc, in_maps, sim=False):
    if sim:
        from concourse.bass_interp import MultiCoreSim
        s = MultiCoreSim(nc, NCORES)
        for c in range(NCORES):
            for k, v in in_maps[c].items():
                s.cores[c].tensor(k)[:] = v
        s.simulate()
        outs = []
        for c in range(NCORES):
            outs.append({a.memorylocations[0].name:
                         np.array(s.cores[c].mem_tensor(a.memorylocations[0].name))
                         for a in nc.m.functions[0].allocations
                         if getattr(a, "kind", None) == "ExternalOutput"})
        return outs
    trace = bool(int(os.environ.get("KNN_TRACE", "0")))
    if trace:
        try:
            res = run_bass_kernel_spmd(nc, in_maps, list(range(NCORES)),
                                       trace=True)
        except Exception:
            res = run_bass_kernel_spmd(nc, in_maps, list(range(NCORES)))
    else:
        res = run_bass_kernel_spmd(nc, in_maps, list(range(NCORES)))
    if res.exec_time_ns is not None:
        LAST_EXEC_PARTS.append(res.exec_time_ns)
    return res.results


def kernel(fts_gt, fts1, W1, W2, k=-1, _cfg=None, _sim=False):
    global LAST_EXEC_NS
    LAST_EXEC_PARTS.clear()
    cfg = _cfg or CFG
    N = cfg.N
    nc_a, nc_b = _get_programs(cfg)

    xs = [np.asarray(fts_gt, np.float32), np.asarray(fts1, np.float32)]
    W1 = np.asarray(W1, np.float32)
    W2 = np.asarray(W2, np.float32)

    # ---- launch A ----
    xT16 = [np.ascontiguousarray(x.astype(np.float16).T) for x in xs]
    in_a = []
    for core in range(NCORES):
        s, q = core // NQ, core % NQ
        in_a.append({
            "xt": xT16[s],
            "xmyt": np.ascontiguousarray(
                xT16[s][:, q * cfg.RQ:(q + 1) * cfg.RQ]),
            "w1s": np.ascontiguousarray(
                W1[:, q * cfg.SL:(q + 1) * cfg.SL]).astype(np.float16),
        })
    out_a = _run(nc_a, in_a, sim=_sim)

    idx_side = []
    a1_side = []
    for s in range(2):
        idx = np.concatenate(
            [out_a[s * NQ + q]["idx_out"] for q in range(NQ)], axis=0)
        idx_side.append(idx[:, :15].astype(np.int64))
        a1_side.append([out_a[s * NQ + q]["a1_out"] for q in range(NQ)])

    # ---- host bookkeeping ----
    iota_f = np.tile(np.arange(128, dtype=np.float32)[None, :],
                     (128, 1)).astype(np.float16)
    lists = [host_lists(cfg, idx_side[s]) for s in range(2)]

    in_b = []
    for core in range(NCORES):
        s, v = core // NQ, core % NQ
        L = lists[s]
        z1 = np.zeros((N + 1, cfg.SL), np.float16)
        z1[:N] = a1_side[s][v]
        w2t = np.ascontiguousarray(
            W2[v * cfg.SL:(v + 1) * cfg.SL, :].T).astype(np.float16)
        in_b.append({
            "z1": z1, "gidx": L["gidx"], "gw": L["gw"], "umask": L["umask"],
            "ssrc": L["ssrc"], "sdst": L["sdst"], "iota_f": iota_f,
            "w2t": w2t, "bperm": L["bperm"],
        })
    out_b = _run(nc_b, in_b, sim=_sim)

    if LAST_EXEC_PARTS:
        LAST_EXEC_NS = float(sum(LAST_EXEC_PARTS))
    parts = [float(out_b[c]["res"][0, 0]) for c in range(NCORES)]
    y0 = np.float32(sum(parts[:NQ]) / N)
    y1 = np.float32(sum(parts[NQ:]) / N)
    sig = np.asarray(
        [1.0 / (1.0 + np.exp(-(np.float32(y0 - y1))))], np.float32)
    return (sig, y0, y1)


# revision 16
# speedup vs baseline: 1.1206x; 1.1206x over previous
"""Trainium2 Bass kernel for nn_DiffRankNet (retrieval_knn).

Strategy (8 NeuronCores, zero mid-kernel feature collectives):
  Launch A  (core = (side s, quarter q)): fp16 cdist score rows for the
    core's N/4 nodes + exact top-16 via max8/max_index/match_replace, plus
    the A1 = x @ W1[:, q-slice] projection.  Outputs per core: knn indices
    [N/4, 16] and A1 slice [N, HID/4].
  Host: pure index bookkeeping — degree vectors, gather lists, one-hot
    weights, balanced dst-tiles, the topology vector b~ (layer 2 of the
    HGNN collapses to b~^T relu(raw) w2s because the final output only
    needs mean().sum()).
  Launch B  (core = (side s, hid-slice v)): dma_gather edge neighborhoods
    of A1-slice rows, prefix-sum matmuls -> c rows, gather c rows by
    dst-sorted membership lists, one-hot matmuls accumulate raw out1^T
    tiles, relu, dot with w2s and b~, one tiny [1,2] AllReduce + sigmoid.
"""

import os
from contextlib import ExitStack
from dataclasses import dataclass

import numpy as np
import ml_dtypes

import concourse.bass as bass
import concourse.mybir as mybir
from concourse import bacc
import concourse.tile as tile
from concourse import library_config
from concourse.bass_utils import run_bass_kernel_spmd

f32 = mybir.dt.float32
bf16 = mybir.dt.bfloat16
fp16 = mybir.dt.float16
i16 = mybir.dt.int16
u32 = mybir.dt.uint32

KS = (5, 10, 15)
NQ = 4          # quarters / hid-slices per side
NCORES = 8
PAD_SLOT = 15   # 16th gather slot per triple is padding
GS = 16         # gather slots per triple


@dataclass(frozen=True)
class Cfg:
    N: int = 4096
    C: int = 1024
    HID: int = 512
    FEAT: int = 128

    @property
    def RQ(self):  # cdist rows per core
        return self.N // NQ

    @property
    def RT(self):  # cdist row-tile (partition) size
        return min(128, self.RQ)

    @property
    def SL(self):  # hid cols per slice-core
        return self.HID // NQ

    @property
    def KC(self):  # contraction chunks of x
        return self.C // 128

    @property
    def CCW(self):  # cdist psum col-tile width
        return min(512, self.N)

    @property
    def G1(self):  # stage-1 gather entries
        return self.N * GS

    @property
    def B1(self):  # stage-1 gather batch (entries; HW-safe dma_gather size)
        return min(1024, self.G1)

    @property
    def CROWS(self):
        return self.N * 4

    @property
    def S2T(self):  # dst tiles
        return self.N // 128

    @property
    def KCH(self):  # scatter chunks per dst tile
        return 17

    @property
    def S2N(self):
        return self.S2T * self.KCH * 128


CFG = Cfg()


# --------------------------------------------------------------------------
# Launch A: cdist + topk + A1 slice
# --------------------------------------------------------------------------
def build_launch_a(cfg: Cfg):
    nc = bacc.Bacc("TRN2", target_bir_lowering=False, debug=False,
                   num_devices=NCORES)
    N, C, KC, RT, CCW = cfg.N, cfg.C, cfg.KC, cfg.RT, cfg.CCW
    IT = cfg.RQ // RT          # row tiles per core
    CCn = N // CCW             # cdist col tiles
    OT = N // 128              # A1 row tiles

    xt_d = nc.dram_tensor("xt", [C, N], fp16, kind="ExternalInput")
    xmyt_d = nc.dram_tensor("xmyt", [C, cfg.RQ], fp16, kind="ExternalInput")
    w1s_d = nc.dram_tensor("w1s", [C, cfg.SL], fp16, kind="ExternalInput")
    idx_o = nc.dram_tensor("idx_out", [cfg.RQ, 16], u32, kind="ExternalOutput")
    a1_o = nc.dram_tensor("a1_out", [N, cfg.SL], fp16, kind="ExternalOutput")

    with tile.TileContext(nc) as tc, ExitStack() as ctx:
        xt_p = ctx.enter_context(tc.tile_pool(name="xt", bufs=KC))
        xmy_p = ctx.enter_context(tc.tile_pool(name="xmy", bufs=KC))
        w1_p = ctx.enter_context(tc.tile_pool(name="w1", bufs=KC))
        const_p = ctx.enter_context(tc.tile_pool(name="const", bufs=1))
        sq_p = ctx.enter_context(tc.tile_pool(name="sq", bufs=2))
        d2_p = ctx.enter_context(tc.tile_pool(name="d2", bufs=2))
        top_p = ctx.enter_context(tc.tile_pool(name="top", bufs=3))
        a1s_p = ctx.enter_context(tc.tile_pool(name="a1s", bufs=4))
        d2ps_p = ctx.enter_context(tc.tile_pool(name="d2ps", bufs=6, space="PSUM"))
        aux_ps = ctx.enter_context(tc.tile_pool(name="auxps", bufs=2, space="PSUM"))

        xt = []
        xmy = []
        w1 = []
        for kc in range(KC):
            t = xt_p.tile([128, N], fp16)
            nc.sync.dma_start(t[:], xt_d.ap()[kc * 128:(kc + 1) * 128, :])
            xt.append(t)
            t2 = xmy_p.tile([128, cfg.RQ], fp16)
            nc.sync.dma_start(t2[:], xmyt_d.ap()[kc * 128:(kc + 1) * 128, :])
            xmy.append(t2)
            t3 = w1_p.tile([128, cfg.SL], fp16)
            nc.sync.dma_start(t3[:], w1s_d.ap()[kc * 128:(kc + 1) * 128, :])
            w1.append(t3)

        ones_col = const_p.tile([128, 1], fp16)
        nc.vector.memset(ones_col[:], 1.0)
        auglhs = const_p.tile([128, 128], fp16)
        nc.vector.memset(auglhs[:], 0.0)
        nc.vector.memset(auglhs[0:2, :], 1.0)
        aug = const_p.tile([128, N], fp16)
        nc.vector.memset(aug[:], 0.0)

        # n_j rows (hi/lo of -n/2) folded into the contraction
        for cc in range(CCn):
            n_ps = aux_ps.tile([1, CCW], f32, tag="aux")
            for kc in range(KC):
                sq = sq_p.tile([128, CCW], fp16)
                nc.scalar.square(sq[:], xt[kc][:, cc * CCW:(cc + 1) * CCW])
                nc.tensor.matmul(n_ps[:], lhsT=ones_col[:], rhs=sq[:],
                                 start=(kc == 0), stop=(kc == KC - 1))
            hi_t = sq_p.tile([1, CCW], fp16, tag="hilo")
            lo_t = sq_p.tile([1, CCW], fp16, tag="hilo")
            nc.scalar.mul(hi_t[:], n_ps[:], -0.5)
            nc.vector.scalar_tensor_tensor(
                out=lo_t[:], in0=n_ps[:], scalar=-0.5, in1=hi_t[:],
                op0=mybir.AluOpType.mult, op1=mybir.AluOpType.subtract)
            nc.sync.dma_start(aug[0:1, cc * CCW:(cc + 1) * CCW], hi_t[:])
            nc.sync.dma_start(aug[1:2, cc * CCW:(cc + 1) * CCW], lo_t[:])

        # cdist + topk per row tile.  kc-outer over cc-halves so the
        # stationary operand loads once per (it, kc, half) instead of per
        # (it, cc, kc); the half's psum tiles accumulate in parallel.
        HCC = max(1, CCn // 2)
        for it in range(IT):
            d2 = d2_p.tile([RT, N], f32)
            for h in range(CCn // HCC if CCn >= HCC else 1):
                pss = []
                for _ci in range(HCC):
                    ps_t = d2ps_p.tile([RT, CCW], f32, tag="d2ps")
                    pss.append(ps_t)
                for kc in range(KC):
                    for ci in range(HCC):
                        cc = h * HCC + ci
                        nc.tensor.matmul(
                            pss[ci][:],
                            lhsT=xmy[kc][:, it * RT:(it + 1) * RT],
                            rhs=xt[kc][:, cc * CCW:(cc + 1) * CCW],
                            start=(kc == 0), stop=False)
                for ci in range(HCC):
                    cc = h * HCC + ci
                    nc.tensor.matmul(
                        pss[ci][:], lhsT=auglhs[:, :RT],
                        rhs=aug[:, cc * CCW:(cc + 1) * CCW],
                        start=False, stop=True)
                    nc.scalar.copy(d2[:, cc * CCW:(cc + 1) * CCW],
                                   pss[ci][:])
            mx1 = top_p.tile([RT, 8], f32, tag="mx")
            nc.vector.max(out=mx1[:], in_=d2[:])
            ix1 = top_p.tile([RT, 8], u32, tag="ix")
            nc.vector.max_index(ix1[:], mx1[:], d2[:])
            nc.vector.match_replace(out=d2[:], in_to_replace=mx1[:],
                                    in_values=d2[:], imm_value=-3.0e38)
            mx2 = top_p.tile([RT, 8], f32, tag="mx")
            nc.vector.max(out=mx2[:], in_=d2[:])
            ix2 = top_p.tile([RT, 8], u32, tag="ix")
            nc.vector.max_index(ix2[:], mx2[:], d2[:])
            idxt = top_p.tile([RT, 16], u32, tag="idxt")
            nc.vector.tensor_copy(idxt[:, 0:8], ix1[:])
            nc.vector.tensor_copy(idxt[:, 8:16], ix2[:])
            nc.sync.dma_start(idx_o.ap()[it * RT:(it + 1) * RT, :], idxt[:])

        # A1 slice
        for ot in range(OT):
            aps = aux_ps.tile([128, cfg.SL], f32, tag="aux")
            for kc in range(KC):
                nc.tensor.matmul(aps[:],
                                 lhsT=xt[kc][:, ot * 128:(ot + 1) * 128],
                                 rhs=w1[kc][:],
                                 start=(kc == 0), stop=(kc == KC - 1))
            a1sb = a1s_p.tile([128, cfg.SL], fp16)
            nc.scalar.copy(a1sb[:], aps[:])
            nc.sync.dma_start(a1_o.ap()[ot * 128:(ot + 1) * 128, :], a1sb[:])

    nc.compile()
    return nc


# --------------------------------------------------------------------------
# Launch B: hyconv sandwich on A1 slices + scalar tail
# --------------------------------------------------------------------------
def build_launch_b(cfg: Cfg):
    nc = bacc.Bacc("TRN2", target_bir_lowering=False, debug=False,
                   num_devices=NCORES)
    N, SL = cfg.N, cfg.SL
    G1, B1 = cfg.G1, cfg.B1
    NB1 = G1 // B1                 # stage-1 gather batches
    NCH1 = B1 // 128               # chunks per stage-1 batch
    NBG1 = NCH1 // 4               # psum bank groups per batch
    RPB = B1 // 4                  # c rows per batch
    S2T, KCH = cfg.S2T, cfg.KCH
    TPB2 = 2 if S2T >= 2 else 1    # dst tiles per stage-2 gather batch
    GB2 = TPB2 * KCH * 128
    NB2 = S2T // TPB2

    z1_d = nc.dram_tensor("z1", [N + 1, SL], fp16, kind="ExternalInput")
    gidx_d = nc.dram_tensor("gidx", [128, G1 // 16], i16, kind="ExternalInput")
    gw_d = nc.dram_tensor("gw", [128, G1 // 128], f32, kind="ExternalInput")
    umask_d = nc.dram_tensor("umask", [128, 32], fp16, kind="ExternalInput")
    ssrc_d = nc.dram_tensor("ssrc", [128, cfg.S2N // 16], i16, kind="ExternalInput")
    sdst_d = nc.dram_tensor("sdst", [128, S2T * KCH], f32, kind="ExternalInput")
    iota_d = nc.dram_tensor("iota_f", [128, 128], fp16, kind="ExternalInput")
    w2t_d = nc.dram_tensor("w2t", [cfg.FEAT, 128], fp16, kind="ExternalInput")
    bperm_d = nc.dram_tensor("bperm", [1, N], f32, kind="ExternalInput")

    res_o = nc.dram_tensor("res", [1, 1], f32, kind="ExternalOutput")

    c_hbm = nc.dram_tensor("c_hbm", [cfg.CROWS + 1, 128], fp16)

    with tile.TileContext(nc) as tc, ExitStack() as ctx:
        const_p = ctx.enter_context(tc.tile_pool(name="const", bufs=1))
        g1_p = ctx.enter_context(tc.tile_pool(name="g1", bufs=3))
        t1w_p = ctx.enter_context(tc.tile_pool(name="t1w", bufs=3))
        stg_p = ctx.enter_context(tc.tile_pool(name="stg", bufs=3))
        g2_p = ctx.enter_context(tc.tile_pool(name="g2", bufs=4))
        t2_p = ctx.enter_context(tc.tile_pool(name="t2", bufs=6))
        rl_p = ctx.enter_context(tc.tile_pool(name="rl", bufs=3))
        fin_p = ctx.enter_context(tc.tile_pool(name="fin", bufs=1))
        cps_p = ctx.enter_context(tc.tile_pool(name="cps", bufs=3, space="PSUM"))
        ops_p = ctx.enter_context(tc.tile_pool(name="ops", bufs=3, space="PSUM"))
        aux_ps = ctx.enter_context(tc.tile_pool(name="auxps", bufs=2, space="PSUM"))

        nc.gpsimd.load_library(library_config.mlp)

        # constant-ish inputs
        gidx = const_p.tile([128, G1 // 16], i16)
        nc.sync.dma_start(gidx[:], gidx_d.ap())
        gw = const_p.tile([128, G1 // 128], f32)
        nc.sync.dma_start(gw[:], gw_d.ap())
        umask = const_p.tile([128, 32], fp16)
        nc.sync.dma_start(umask[:], umask_d.ap())
        ssrc = const_p.tile([128, cfg.S2N // 16], i16)
        nc.sync.dma_start(ssrc[:], ssrc_d.ap())
        sdst = const_p.tile([128, S2T * KCH], f32)
        nc.sync.dma_start(sdst[:], sdst_d.ap())
        iota = const_p.tile([128, 128], fp16)
        nc.sync.dma_start(iota[:], iota_d.ap())
        w2t = const_p.tile([cfg.FEAT, 128], fp16)
        nc.sync.dma_start(w2t[:], w2t_d.ap())
        bperm = const_p.tile([1, N], f32)
        nc.sync.dma_start(bperm[:], bperm_d.ap())

        ones_col = const_p.tile([128, 1], fp16)
        nc.vector.memset(ones_col[:], 1.0)

        # zero pad row of c storage
        zr = const_p.tile([1, 128], fp16)
        nc.vector.memset(zr[:], 0.0)
        nc.sync.dma_start(c_hbm.ap()[cfg.CROWS:cfg.CROWS + 1, :], zr[:])

        # w2s column = W2[v-slice, :] @ 1  (lhsT = W2T slice)
        w2s_ps = aux_ps.tile([128, 1], f32, tag="aux")
        nc.tensor.matmul(w2s_ps[:], lhsT=w2t[:], rhs=ones_col[:cfg.FEAT, :],
                         start=True, stop=True)
        w2s = const_p.tile([128, 1], fp16)
        nc.vector.tensor_copy(w2s[:], w2s_ps[:])

        # ---- stage 1: edge-neighborhood gather + prefix-sum matmuls ----
        for b in range(NB1):
            g = g1_p.tile([128, NCH1, 128], fp16)
            nc.gpsimd.dma_gather(
                out_ap=g[:], in_ap=z1_d.ap(),
                idxs_ap=gidx[:, b * (B1 // 16):(b + 1) * (B1 // 16)],
                num_idxs=B1, num_idxs_reg=B1, elem_size=128)
            t1w = t1w_p.tile([128, NCH1, 32], fp16)
            nc.vector.tensor_tensor(
                out=t1w[:],
                in0=umask[:].rearrange("p (c m) -> p c m", c=1)
                    .to_broadcast([128, NCH1, 32]),
                in1=gw[:, b * NCH1:(b + 1) * NCH1]
                    .rearrange("p (c m) -> p c m", m=1)
                    .to_broadcast([128, NCH1, 32]),
                op=mybir.AluOpType.mult)
            stg = stg_p.tile([128, NBG1 * 128], fp16)
            for bg in range(NBG1):
                cps = cps_p.tile([128, 128], f32)
                for j in range(4):
                    c = bg * 4 + j
                    nc.tensor.matmul(
                        cps[32 * j:32 * j + 32, :],
                        lhsT=t1w[:, c, :], rhs=g[:, c, :],
                        start=True, stop=True,
                        tile_position=(0, 32 * j))
                nc.scalar.copy(stg[:, bg * 128:(bg + 1) * 128], cps[:])
            nc.sync.dma_start(
                c_hbm.ap()[b * RPB:(b + 1) * RPB, :]
                    .rearrange("(g p) f -> p g f", p=128),
                stg[:].rearrange("p (g f) -> p g f", f=128))

        # ---- stage 2: dst-sorted membership gather + one-hot matmuls ----
        yacc = fin_p.tile([1, S2T], f32)
        yscr = fin_p.tile([1, 128], f32)
        S2CH = S2T * KCH
        GCH = 8                     # chunks per gather batch (1024 idxs)
        g2 = None
        ops = None
        for ch in range(S2CH):
            if ch % GCH == 0:
                nch = min(GCH, S2CH - ch)
                g2 = g2_p.tile([128, GCH, 128], fp16, tag="g2")
                nc.gpsimd.dma_gather(
                    out_ap=g2[:, 0:nch, :], in_ap=c_hbm.ap(),
                    idxs_ap=ssrc[:, ch * 8:(ch + nch) * 8],
                    num_idxs=nch * 128, num_idxs_reg=nch * 128,
                    elem_size=128)
            d, j = divmod(ch, KCH)
            if j == 0:
                ops = ops_p.tile([128, 128], f32, tag="ops")
            t2 = t2_p.tile([128, 128], fp16)
            nc.vector.tensor_scalar(
                out=t2[:], in0=iota[:],
                scalar1=sdst[:, ch:ch + 1], scalar2=None,
                op0=mybir.AluOpType.is_equal)
            nc.tensor.matmul(ops[:], lhsT=g2[:, ch % GCH, :], rhs=t2[:],
                             start=(j == 0), stop=(j == KCH - 1))
            if j == KCH - 1:
                rl = rl_p.tile([128, 128], fp16)
                nc.scalar.activation(rl[:], ops[:],
                                     mybir.ActivationFunctionType.Relu)
                rps = aux_ps.tile([1, 128], f32, tag="aux")
                nc.tensor.matmul(rps[:], lhsT=w2s[:], rhs=rl[:],
                                 start=True, stop=True)
                nc.vector.scalar_tensor_tensor(
                    out=yscr[:], in0=rps[:], scalar=1.0,
                    in1=bperm[:, d * 128:(d + 1) * 128],
                    op0=mybir.AluOpType.mult, op1=mybir.AluOpType.mult,
                    accum_out=yacc[0:1, d:d + 1])

        # ---- tail: per-core partial scalar out (host combines) ----
        ysum = fin_p.tile([1, 1], f32)
        nc.vector.tensor_reduce(out=ysum[:], in_=yacc[:],
                                axis=mybir.AxisListType.X,
                                op=mybir.AluOpType.add)
        nc.sync.dma_start(res_o.ap(), ysum[:])

    nc.compile()
    return nc


# --------------------------------------------------------------------------
# Host-side index bookkeeping between launches
# --------------------------------------------------------------------------
def _wrap16(arr):
    """idx j -> [j % 16, j // 16], tiled to 128 partitions."""
    a = np.asarray(arr, dtype=np.int16).reshape(-1, 16).T
    return np.tile(a, (8, 1))


def host_lists(cfg: Cfg, idx):
    """idx: [N, 15] int64 neighbor indices (rank order). Returns dict of
    launch-B input arrays (except z1/w2t/smask) for one side."""
    N = cfg.N
    K0, K1, K2 = KS
    r = np.arange(15)
    # degree vectors
    Dv = np.zeros(N, np.int64)
    for k in KS:
        Dv += np.bincount(idx[:, :k].ravel(), minlength=N)
    dv = (1.0 / np.sqrt(Dv)).astype(np.float64)

    # u_a(r) = sum_{lvl >= a, KS[lvl] > r} 1/KS[lvl]
    u = np.zeros((3, 16), np.float64)
    for a in range(3):
        for rr in range(15):
            u[a, rr] = sum(1.0 / k for li, k in enumerate(KS)
                           if li >= a and k > rr)
    # umask [128, 32]
    umask = np.zeros((128, 32), np.float64)
    for p in range(128):
        t, rr = p // 16, p % 16
        if rr < 15:
            for a in range(3):
                umask[p, t * 4 + a] = u[a, rr]
    umask = umask.astype(np.float16)

    # stage-1 gather list + per-entry dv weight
    gl = np.full((N, GS), N, np.int64)
    gl[:, :15] = idx[:, :15]
    gidx = _wrap16(gl.ravel())
    gwv = np.zeros(N * GS, np.float64)
    gwv.reshape(N, GS)[:, :15] = dv[idx[:, :15]]
    gw = gwv.reshape(-1, 128).T.astype(np.float32)  # [128, G1//128]

    # c-row id for (e, a)
    TPB = cfg.B1 // 16
    def crow(e, a):
        eb, el = e // TPB, e % TPB
        c = el // 8
        return eb * (TPB * 4) + (c // 4) * 128 + (c % 4) * 32 + (e % 8) * 4 + a

    # b~ = (H g) * dv^2 with g = de * (H^T dv)
    S = np.add.reduceat(dv[idx[:, :15]], [0, K0, K1], axis=1).cumsum(1)
    # S[:, l] = sum_{r < KS[l]} dv[idx]; columns: [0:K0], [K0:K1], [K1:15]
    glev = S / np.array(KS, np.float64)[None, :]     # g_(e,lvl)
    gcomb = np.zeros((N, 15), np.float64)
    for rr in range(15):
        lvls = [li for li, k in enumerate(KS) if k > rr]
        gcomb[:, rr] = glev[:, lvls].sum(1)
    Hg = np.bincount(idx[:, :15].ravel(), weights=gcomb.ravel(), minlength=N)
    bt = Hg * dv * dv

    # balanced dst tiles: greedy LPT under exact-128 capacity
    mcnt = np.bincount(idx[:, :15].ravel(), minlength=N)
    order = np.argsort(-mcnt, kind="stable")
    S2T = cfg.S2T
    tile_of = np.zeros(N, np.int64)
    loc_of = np.zeros(N, np.int64)
    tl_fill = np.zeros(S2T, np.int64)
    tl_cnt = np.zeros(S2T, np.int64)
    INF = 1 << 60
    cost = np.zeros(S2T, np.int64)
    for node in order:
        d = int(np.argmin(np.where(tl_fill < 128, tl_cnt, INF)))
        tile_of[node] = d
        loc_of[node] = tl_fill[d]
        tl_fill[d] += 1
        tl_cnt[d] += mcnt[node]
    assert (tl_fill == 128).all()
    cap = cfg.KCH * 128
    assert tl_cnt.max() <= cap, f"dst tile overflow: {tl_cnt.max()} > {cap}"

    # membership entries
    e_arr = np.repeat(np.arange(N), 15)
    r_arr = np.tile(np.arange(15), N)
    dst = idx[:, :15].ravel()
    buck = (r_arr >= K0).astype(np.int64) + (r_arr >= K1)
    src = crow(e_arr, buck)
    dtile = tile_of[dst]
    dloc = loc_of[dst]

    ssrc_full = np.full(cfg.S2N, cfg.CROWS, np.int64)
    sdst_full = np.zeros(cfg.S2N, np.int64)
    for d in range(S2T):
        sel = np.flatnonzero(dtile == d)
        base = d * cap
        ssrc_full[base:base + sel.size] = src[sel]
        sdst_full[base:base + sel.size] = dloc[sel]
    # wrap ssrc per 1024-idx gather batch (ragged tail allowed)
    GB2 = 1024
    parts = [_wrap16(ssrc_full[g:g + GB2])
             for g in range(0, cfg.S2N, GB2)]
    ssrc = np.concatenate(parts, axis=1)
    sdst = sdst_full.reshape(-1, 128).T.astype(np.float32)  # [128, S2T*KCH]

    # b~ permuted to (tile, local) order
    bperm = np.zeros(N, np.float64)
    bperm[tile_of * 128 + loc_of] = bt
    return dict(gidx=gidx, gw=gw, umask=umask, ssrc=ssrc.astype(np.int16),
                sdst=sdst, bperm=bperm.astype(np.float32)[None, :])


# --------------------------------------------------------------------------
# Top-level kernel
# --------------------------------------------------------------------------
_NC_CACHE = {}
LAST_EXEC_NS = None
LAST_EXEC_PARTS = []


def _get_programs(cfg: Cfg):
    key = (cfg.N, cfg.C, cfg.HID, cfg.FEAT)
    if key not in _NC_CACHE:
        _NC_CACHE[key] = (build_launch_a(cfg), build_launch_b(cfg))
    return _NC_CACHE[key]


def _run(nc, in_maps, sim=False):
    if sim:
        from concourse.bass_interp import MultiCoreSim
        s = MultiCoreSim(nc, NCORES)
        for c in range(NCORES):
            for k, v in in_maps[c].items():
                s.cores[c].tensor(k)[:] = v
        s.simulate()
        outs = []
        for c in range(NCORES):
            outs.append({a.memorylocations[0].name:
                         np.array(s.cores[c].mem_tensor(a.memorylocations[0].name))
                         for a in nc.m.functions[0].allocations
                         if getattr(a, "kind", None) == "ExternalOutput"})
        return outs
    trace = bool(int(os.environ.get("KNN_TRACE", "0")))
    if trace:
        try:
            res = run_bass_kernel_spmd(nc, in_maps, list(range(NCORES)),
                                       trace=True)
        except Exception:
            res = run_bass_kernel_spmd(nc, in_maps, list(range(NCORES)))
    else:
        res = run_bass_kernel_spmd(nc, in_maps, list(range(NCORES)))
    if res.exec_time_ns is not None:
        LAST_EXEC_PARTS.append(res.exec_time_ns)
    return res.results


def kernel(fts_gt, fts1, W1, W2, k=-1, _cfg=None, _sim=False):
    global LAST_EXEC_NS
    LAST_EXEC_PARTS.clear()
    cfg = _cfg or CFG
    N = cfg.N
    nc_a, nc_b = _get_programs(cfg)

    xs = [np.asarray(fts_gt, np.float32), np.asarray(fts1, np.float32)]
    W1 = np.asarray(W1, np.float32)
    W2 = np.asarray(W2, np.float32)

    # ---- launch A ----
    xT16 = [np.ascontiguousarray(x.astype(np.float16).T) for x in xs]
    in_a = []
    for core in range(NCORES):
        s, q = core // NQ, core % NQ
        in_a.append({
            "xt": xT16[s],
            "xmyt": np.ascontiguousarray(
                xT16[s][:, q * cfg.RQ:(q + 1) * cfg.RQ]),
            "w1s": np.ascontiguousarray(
                W1[:, q * cfg.SL:(q + 1) * cfg.SL]).astype(np.float16),
        })
    out_a = _run(nc_a, in_a, sim=_sim)

    idx_side = []
    a1_side = []
    for s in range(2):
        idx = np.concatenate(
            [out_a[s * NQ + q]["idx_out"] for q in range(NQ)], axis=0)
        idx_side.append(idx[:, :15].astype(np.int64))
        a1_side.append([out_a[s * NQ + q]["a1_out"] for q in range(NQ)])

    # ---- host bookkeeping ----
    iota_f = np.tile(np.arange(128, dtype=np.float32)[None, :],
                     (128, 1)).astype(np.float16)
    lists = [host_lists(cfg, idx_side[s]) for s in range(2)]

    in_b = []
    for core in range(NCORES):
        s, v = core // NQ, core % NQ
        L = lists[s]
        z1 = np.zeros((N + 1, cfg.SL), np.float16)
        z1[:N] = a1_side[s][v]
        w2t = np.ascontiguousarray(
            W2[v * cfg.SL:(v + 1) * cfg.SL, :].T).astype(np.float16)
        in_b.append({
            "z1": z1, "gidx": L["gidx"], "gw": L["gw"], "umask": L["umask"],
            "ssrc": L["ssrc"], "sdst": L["sdst"], "iota_f": iota_f,
            "w2t": w2t, "bperm": L["bperm"],
        })
    out_b = _run(nc_b, in_b, sim=_sim)

    if LAST_EXEC_PARTS:
        LAST_EXEC_NS = float(sum(LAST_EXEC_PARTS))
    parts = [float(out_b[c]["res"][0, 0]) for c in range(NCORES)]
    y0 = np.float32(sum(parts[:NQ]) / N)
    y1 = np.float32(sum(parts[NQ:]) / N)
    sig = np.asarray(
        [1.0 / (1.0 + np.exp(-(np.float32(y0 - y1))))], np.float32)
    return (sig, y0, y1)
